# revision 20
# baseline (speedup 1.0000x reference)
"""Grouped-experts SwiGLU MoE kernel for Trainium2 (8 NeuronCores).

Problem: x [8192, 2048] f32, 8 experts with w1/w3 [8, 1408, 2048] and
w2 [8, 2048, 1408]; tokens are expert-contiguous with a per-expert count
vector. out[t] = (silu(x_t @ w1_e.T) * (x_t @ w3_e.T)) @ w2_e.T for the
expert e owning token t.

Sharding: pure expert parallelism. Core e receives expert e's 1024-token
tile (dynamic-slice semantics of the reference) plus expert e's weights,
and computes the full SwiGLU MLP for that tile. No collectives.

Design (PE-roofline focused; per-core floor is 1056 matmuls x 512 cyc /
2.4 GHz = 225.3 us):
  - All matmul operands are bf16 (PSUM accumulates f32): same 1 cycle/row
    PE rate as f32r at free-dim 512, but half the HBM traffic. Measured
    end-to-end rel err ~3.6e-3 vs the f32 reference (gate is 2e-2).
  - One DGE queue (SP): descriptor generation and the transfer engine are
    effectively serial, so DMA *issue order* is the schedule. Head order:
    w1[0]/w3[0] in halves woven between the first x chunks, then the rest
    of x (16 chunks of [128, 2, 512] = 256 KB, 1 KB partition lines), then
    the remaining weight pairs (slot-throttled by wpool bufs), then all of
    w2. Output stores are issued last and have the queue to themselves.
  - A short warmup burst of dummy matmuls (dep: one memset) runs during
    the head so the PE clock ramp completes before real work arrives.
  - Stage 1 interleaves the w1/w3 accumulations per k (emission order
    matched to chunk arrival for the first block); silu (ACT) + mul (DVE)
    write hT in bf16. Stage 2 keeps hT stationary per token-tile and
    streams w2; the final group is split into 4 column sub-groups so its
    copies/stores pipeline under the sub-groups' matmuls.
"""

from contextlib import ExitStack

import numpy as np

import concourse.bass as bass
import concourse.mybir as mybir
import concourse.tile as tile
from concourse import bacc
from concourse.bass import ts
from concourse.bass_utils import run_bass_kernel_spmd

F32 = mybir.dt.float32
BF16 = mybir.dt.bfloat16

N_TOKENS = 8192
DIM = 2048
HIDDEN = 1408
N_EXPERTS = 8
CAP = N_TOKENS // N_EXPERTS  # 1024 tokens per core
P = 128
KD = DIM // P  # 16 contraction tiles, stage 1
KD2 = KD // 2  # x DMA chunks pair two k-tiles for 1KB partition lines
KH = HIDDEN // P  # 11 contraction tiles, stage 2
TB = 512  # token-block (stage-1 moving free dim)
DB = 512  # dim-block (stage-2 moving free dim)
N_TB = CAP // TB  # 2
N_DB = DIM // DB  # 4
N_TT = CAP // P  # 8 token tiles (stage-2 stationary)
N_WARM = 6  # warmup matmuls: enough to finish the clock ramp

_CACHED_NC = None


def _build_nc(repeat=1):
    """Build the kernel; repeat>1 chains `repeat` serialized copies of the
    body (each repeat's input DMAs gated on the previous repeat's final
    store) so wall-clock slope over `repeat` measures one execution."""
    import bass_rust

    nc = bacc.Bacc("TRN2", debug=False)
    xQ = nc.dram_tensor("xQ", [N_TB, KD2, P, 2, TB], BF16, kind="ExternalInput").ap()
    w1Q = nc.dram_tensor("w1Q", [KH, P, KD, P], BF16, kind="ExternalInput").ap()
    w3Q = nc.dram_tensor("w3Q", [KH, P, KD, P], BF16, kind="ExternalInput").ap()
    w2Q = nc.dram_tensor("w2Q", [N_DB, P, KH, DB], BF16, kind="ExternalInput").ap()
    out = nc.dram_tensor("out", [CAP, DIM], F32, kind="ExternalOutput").ap()
    warm_out = nc.dram_tensor("warm_out", [P, 4], F32, kind="ExternalOutput").ap()

    with tile.TileContext(nc) as tc, ExitStack() as ctx:
        xpool = ctx.enter_context(tc.tile_pool(name="xpool", bufs=1))
        hpool = ctx.enter_context(tc.tile_pool(name="hpool", bufs=1))
        wpool = ctx.enter_context(tc.tile_pool(name="wpool", bufs=3))
        w2pool = ctx.enter_context(tc.tile_pool(name="w2pool", bufs=4))
        warmpool = ctx.enter_context(tc.tile_pool(name="warmpool", bufs=1))
        tmppool = ctx.enter_context(tc.tile_pool(name="tmppool", bufs=3))
        opool = ctx.enter_context(tc.tile_pool(name="opool", bufs=3))
        pspool = ctx.enter_context(tc.tile_pool(name="pspool", bufs=2, space="PSUM"))

        prev_fence = None
        for rep in range(repeat):
            in_dmas = []

            def track(inst):
                in_dmas.append(inst)
                return inst

            x_sb = xpool.tile([P, KD, N_TB, TB], BF16)

            def dma_x(tb, k2):
                track(nc.sync.dma_start(x_sb[:, 2 * k2 : 2 * k2 + 2, tb], xQ[tb, k2]))

            # ---- DMA issue order defines the head schedule ----
            w1_sbs, w3_sbs = [], []
            for ht in range(KH):
                w1_sb = wpool.tile([P, KD, P], BF16, tag="w1")
                w3_sb = wpool.tile([P, KD, P], BF16, tag="w3")
                if ht == 0:
                    # First pair in halves, woven with the first x chunks so
                    # the first ps1 accumulations can issue while w3 streams.
                    H2 = KD // 2
                    track(nc.sync.dma_start(w1_sb[:, 0:H2], w1Q[0, :, 0:H2]))
                    dma_x(0, 0)
                    dma_x(0, 1)
                    track(nc.sync.dma_start(w3_sb[:, 0:H2], w3Q[0, :, 0:H2]))
                    dma_x(0, 2)
                    dma_x(0, 3)
                    track(nc.sync.dma_start(w1_sb[:, H2:KD], w1Q[0, :, H2:KD]))
                    track(nc.sync.dma_start(w3_sb[:, H2:KD], w3Q[0, :, H2:KD]))
                    for k2 in range(4, KD2):
                        dma_x(0, k2)
                    for k2 in range(KD2):
                        dma_x(1, k2)
                else:
                    track(nc.sync.dma_start(w1_sb[:], w1Q[ht]))
                    track(nc.sync.dma_start(w3_sb[:], w3Q[ht]))
                w1_sbs.append(w1_sb)
                w3_sbs.append(w3_sb)
            # All of w2 (5.8 MB): queued behind stage-1 weights, done long
            # before stage 2 begins.
            w2_sbs = []
            for db in range(N_DB):
                w2_sb = w2pool.tile([P, KH, DB], BF16, tag="w2")
                track(nc.sync.dma_start(w2_sb[:], w2Q[db]))
                w2_sbs.append(w2_sb)

            if prev_fence is not None:
                for inst in in_dmas:
                    bass_rust.add_dep_helper(
                        inst.ins, prev_fence.ins, reason="serialize benchmark repeats"
                    )

            if rep == 0:
                # ---- PE warmup: ramp the clock while the head DMAs land ----
                warm_sb = warmpool.tile([P, TB], BF16)
                nc.vector.memset(warm_sb[:], 0.0)
                ps_w = pspool.tile([P, TB], F32, tag="ps2l")
                for i in range(N_WARM):
                    nc.tensor.matmul(
                        ps_w[:], warm_sb[:, 0:P], warm_sb[:],
                        start=(i == 0), stop=(i == N_WARM - 1),
                    )
                wo = tmppool.tile([P, 4], F32, tag="wo")
                nc.vector.tensor_copy(wo[:], ps_w[:, 0:4])
                nc.sync.dma_start(warm_out[:], wo[:])

            # ---- Stage 1: hT[h, t] = silu(w1 xT) * (w3 xT), bf16 ----
            h_sb = hpool.tile([P, KH, CAP], BF16)
            for ht in range(KH):
                w1_sb, w3_sb = w1_sbs[ht], w3_sbs[ht]
                for tb in range(N_TB):
                    ps1 = pspool.tile([P, TB], F32, tag="ps1")
                    ps3 = pspool.tile([P, TB], F32, tag="ps3")
                    if ht == 0 and tb == 0:
                        # Emission order matched to head DMA arrival: ps1 k's
                        # as their x chunks land, ps3 k's after each w3 half.
                        order = (
                            [("1", k) for k in range(4)]
                            + [("3", k) for k in range(4)]
                            + [("1", k) for k in range(4, 8)]
                            + [("3", k) for k in range(4, 8)]
                            + [(g, k) for k in range(8, KD) for g in ("1", "3")]
                        )
                    else:
                        order = [(g, k) for k in range(KD) for g in ("1", "3")]
                    for g, k in order:
                        ps, w_sb = (ps1, w1_sb) if g == "1" else (ps3, w3_sb)
                        nc.tensor.matmul(
                            ps[:], w_sb[:, k], x_sb[:, k, tb],
                            start=(k == 0), stop=(k == KD - 1),
                        )
                    sil = tmppool.tile([P, TB], F32, tag="sil")
                    nc.scalar.activation(
                        sil[:], ps1[:], mybir.ActivationFunctionType.Silu
                    )
                    nc.vector.tensor_mul(h_sb[:, ht, ts(tb, TB)], sil[:], ps3[:])

            # ---- Stage 2: out[t, d] = hT.T @ w2T ----
            for tt in range(N_TT):
                for db in range(N_DB):
                    last = tt == N_TT - 1 and db == N_DB - 1
                    if last:
                        # Split the final group into 4 column sub-groups so
                        # its copies/stores pipeline under the sub-groups'
                        # matmuls instead of serializing a ~2us flush after
                        # the last MM.
                        CW = DB // 4
                        ot = opool.tile([P, DB], F32, tag="ot")
                        for c in range(4):
                            ps = pspool.tile([P, CW], F32, tag="ps2l")
                            for k in range(KH):
                                nc.tensor.matmul(
                                    ps[:],
                                    h_sb[:, k, ts(tt, P)],
                                    w2_sbs[db][:, k, c * CW : (c + 1) * CW],
                                    start=(k == 0), stop=(k == KH - 1),
                                )
                            nc.vector.tensor_copy(ot[:, ts(c, CW)], ps[:])
                            fence = nc.sync.dma_start(
                                out[
                                    ts(tt, P),
                                    db * DB + c * CW : db * DB + (c + 1) * CW,
                                ],
                                ot[:, ts(c, CW)],
                            )
                    else:
                        ps = pspool.tile([P, DB], F32, tag="ps2")
                        for k in range(KH):
                            nc.tensor.matmul(
                                ps[:], h_sb[:, k, ts(tt, P)], w2_sbs[db][:, k],
                                start=(k == 0), stop=(k == KH - 1),
                            )
                        ot = opool.tile([P, DB], F32, tag="ot")
                        nc.vector.tensor_copy(ot[:], ps[:])
                        nc.sync.dma_start(out[ts(tt, P), ts(db, DB)], ot[:])
            prev_fence = fence

    nc.compile()
    return nc


def _get_nc():
    global _CACHED_NC
    if _CACHED_NC is None:
        _CACHED_NC = _build_nc()
    return _CACHED_NC


def _bf16(a):
    return a.astype(mybir.dt.np(mybir.dt.bfloat16))


def _pack_inputs(x, w1, w2, w3, read_starts):
    """Per-core input dicts with DMA-optimal (partition-major) layouts."""
    in_maps = []
    for e in range(N_EXPERTS):
        s = int(read_starts[e])
        xe = x[s : s + CAP]  # [CAP, DIM]
        # xQ[tb, k2, p, j, t] = xe[tb*TB + t, (2*k2 + j)*P + p]
        xQ = np.ascontiguousarray(
            _bf16(xe).T.reshape(KD2, 2, P, N_TB, TB).transpose(3, 0, 2, 1, 4)
        )
        w1Q = np.ascontiguousarray(
            _bf16(w1[e]).T.reshape(KD, P, KH, P).transpose(2, 1, 0, 3)
        )
        w3Q = np.ascontiguousarray(
            _bf16(w3[e]).T.reshape(KD, P, KH, P).transpose(2, 1, 0, 3)
        )
        w2Q = np.ascontiguousarray(
            _bf16(w2[e]).T.reshape(KH, P, N_DB, DB).transpose(2, 1, 0, 3)
        )
        in_maps.append({"xQ": xQ, "w1Q": w1Q, "w3Q": w3Q, "w2Q": w2Q})
    return in_maps


def kernel(x, num_tokens_per_expert, w1, w2, w3):
    x = np.ascontiguousarray(np.asarray(x, dtype=np.float32))
    w1 = np.asarray(w1, dtype=np.float32)
    w2 = np.asarray(w2, dtype=np.float32)
    w3 = np.asarray(w3, dtype=np.float32)
    counts = np.asarray(num_tokens_per_expert).astype(np.int64)

    offsets = np.cumsum(counts)
    starts = offsets - counts
    # jax.lax.dynamic_slice clamps the read start so the slice is in-bounds.
    read_starts = np.clip(starts, 0, N_TOKENS - CAP)

    in_maps = _pack_inputs(x, w1, w2, w3, read_starts)
    nc = _get_nc()
    res = run_bass_kernel_spmd(nc, in_maps, core_ids=list(range(N_EXPERTS)))
    ye = [res.results[e]["out"] for e in range(N_EXPERTS)]

    if np.all(counts == CAP):
        # balanced routing: per-expert tiles are disjoint and exactly cover x
        return np.concatenate(ye, axis=0)

    # general case: mask invalid slots, scatter-add to clipped positions
    y = np.zeros((N_TOKENS, DIM), np.float32)
    slot = np.arange(CAP)
    for e in range(N_EXPERTS):
        valid = slot < counts[e]
        pos = np.clip(starts[e] + slot, 0, N_TOKENS - 1)
        np.add.at(y, pos, np.where(valid[:, None], ye[e], 0.0))
    return y


# revision 21
# speedup vs baseline: 2.5361x; 2.5361x over previous
"""Grouped-experts SwiGLU MoE kernel for Trainium2 (8 NeuronCores).

Problem: x [8192, 2048] f32, 8 experts with w1/w3 [8, 1408, 2048] and
w2 [8, 2048, 1408]; tokens are expert-contiguous with a per-expert count
vector. out[t] = (silu(x_t @ w1_e.T) * (x_t @ w3_e.T)) @ w2_e.T for the
expert e owning token t.

Sharding: pure expert parallelism. Core e receives expert e's 1024-token
tile (dynamic-slice semantics of the reference) plus expert e's weights,
and computes the full SwiGLU MLP for that tile. No collectives.

Design (PE-roofline focused; per-core floor is 1056 matmuls x 512 cyc /
2.4 GHz = 225.3 us):
  - All matmul operands are bf16 (PSUM accumulates f32): same 1 cycle/row
    PE rate as f32r at free-dim 512, but half the HBM traffic. Measured
    end-to-end rel err ~3.6e-3 vs the f32 reference (gate is 2e-2).
  - One DGE queue (SP): descriptor generation and the transfer engine are
    effectively serial, so DMA *issue order* is the schedule. Head order:
    w1[0]/w3[0] in halves woven between the first x chunks, then the rest
    of x (16 chunks of [128, 2, 512] = 256 KB, 1 KB partition lines), then
    the remaining weight pairs (slot-throttled by wpool bufs), then all of
    w2. Output stores are issued last and have the queue to themselves.
  - A short warmup burst of dummy matmuls (dep: one memset) runs during
    the head so the PE clock ramp completes before real work arrives.
  - Stage 1 interleaves the w1/w3 accumulations per k (emission order
    matched to chunk arrival for the first block); silu (ACT) + mul (DVE)
    write hT in bf16. Stage 2 keeps hT stationary per token-tile and
    streams w2; the final group is split into 4 column sub-groups so its
    copies/stores pipeline under the sub-groups' matmuls.
"""

from contextlib import ExitStack

import numpy as np

import concourse.bass as bass
import concourse.mybir as mybir
import concourse.tile as tile
from concourse import bacc
from concourse.bass import ts
from concourse.bass_utils import run_bass_kernel_spmd

F32 = mybir.dt.float32
BF16 = mybir.dt.bfloat16

N_TOKENS = 8192
DIM = 2048
HIDDEN = 1408
N_EXPERTS = 8
CAP = N_TOKENS // N_EXPERTS  # 1024 tokens per core
P = 128
KD = DIM // P  # 16 contraction tiles, stage 1
KD2 = KD // 2  # x DMA chunks pair two k-tiles for 1KB partition lines
KH = HIDDEN // P  # 11 contraction tiles, stage 2
TB = 512  # token-block (stage-1 moving free dim)
DB = 512  # dim-block (stage-2 moving free dim)
N_TB = CAP // TB  # 2
N_DB = DIM // DB  # 4
N_TT = CAP // P  # 8 token tiles (stage-2 stationary)
N_WARM = 6  # warmup matmuls: enough to finish the clock ramp

_CACHED_NC = None


def _build_nc(repeat=1):
    """Build the kernel; repeat>1 chains `repeat` serialized copies of the
    body (each repeat's input DMAs gated on the previous repeat's final
    store) so wall-clock slope over `repeat` measures one execution."""
    import bass_rust

    nc = bacc.Bacc("TRN2", debug=False)
    xQ = nc.dram_tensor("xQ", [N_TB, KD2, P, 2, TB], BF16, kind="ExternalInput").ap()
    w1Q = nc.dram_tensor("w1Q", [KH, P, KD, P], BF16, kind="ExternalInput").ap()
    w3Q = nc.dram_tensor("w3Q", [KH, P, KD, P], BF16, kind="ExternalInput").ap()
    w2Q = nc.dram_tensor("w2Q", [N_DB, P, KH, DB], BF16, kind="ExternalInput").ap()
    out = nc.dram_tensor("out", [CAP, DIM], BF16, kind="ExternalOutput").ap()
    warm_out = nc.dram_tensor("warm_out", [P, 4], F32, kind="ExternalOutput").ap()

    with tile.TileContext(nc) as tc, ExitStack() as ctx:
        xpool = ctx.enter_context(tc.tile_pool(name="xpool", bufs=1))
        hpool = ctx.enter_context(tc.tile_pool(name="hpool", bufs=1))
        wpool = ctx.enter_context(tc.tile_pool(name="wpool", bufs=3))
        w2pool = ctx.enter_context(tc.tile_pool(name="w2pool", bufs=4))
        warmpool = ctx.enter_context(tc.tile_pool(name="warmpool", bufs=1))
        tmppool = ctx.enter_context(tc.tile_pool(name="tmppool", bufs=3))
        opool = ctx.enter_context(tc.tile_pool(name="opool", bufs=3))
        pspool = ctx.enter_context(tc.tile_pool(name="pspool", bufs=2, space="PSUM"))

        prev_fence = None
        for rep in range(repeat):
            in_dmas = []

            def track(inst):
                in_dmas.append(inst)
                return inst

            x_sb = xpool.tile([P, KD, N_TB, TB], BF16)

            def dma_x(tb, k2):
                track(nc.sync.dma_start(x_sb[:, 2 * k2 : 2 * k2 + 2, tb], xQ[tb, k2]))

            # ---- DMA issue order defines the head schedule ----
            w1_sbs, w3_sbs = [], []
            for ht in range(KH):
                w1_sb = wpool.tile([P, KD, P], BF16, tag="w1")
                w3_sb = wpool.tile([P, KD, P], BF16, tag="w3")
                if ht == 0:
                    # First pair in halves, woven with the first x chunks so
                    # the first ps1 accumulations can issue while w3 streams.
                    H2 = KD // 2
                    track(nc.sync.dma_start(w1_sb[:, 0:H2], w1Q[0, :, 0:H2]))
                    dma_x(0, 0)
                    dma_x(0, 1)
                    track(nc.sync.dma_start(w3_sb[:, 0:H2], w3Q[0, :, 0:H2]))
                    dma_x(0, 2)
                    dma_x(0, 3)
                    track(nc.sync.dma_start(w1_sb[:, H2:KD], w1Q[0, :, H2:KD]))
                    track(nc.sync.dma_start(w3_sb[:, H2:KD], w3Q[0, :, H2:KD]))
                    for k2 in range(4, KD2):
                        dma_x(0, k2)
                    for k2 in range(KD2):
                        dma_x(1, k2)
                else:
                    track(nc.sync.dma_start(w1_sb[:], w1Q[ht]))
                    track(nc.sync.dma_start(w3_sb[:], w3Q[ht]))
                w1_sbs.append(w1_sb)
                w3_sbs.append(w3_sb)
            # All of w2 (5.8 MB): queued behind stage-1 weights, done long
            # before stage 2 begins.
            w2_sbs = []
            for db in range(N_DB):
                w2_sb = w2pool.tile([P, KH, DB], BF16, tag="w2")
                track(nc.sync.dma_start(w2_sb[:], w2Q[db]))
                w2_sbs.append(w2_sb)

            if prev_fence is not None:
                for inst in in_dmas:
                    bass_rust.add_dep_helper(
                        inst.ins, prev_fence.ins, reason="serialize benchmark repeats"
                    )

            if rep == 0:
                # ---- PE warmup: ramp the clock while the head DMAs land ----
                warm_sb = warmpool.tile([P, TB], BF16)
                nc.vector.memset(warm_sb[:], 0.0)
                ps_w = pspool.tile([P, TB], F32, tag="ps2l")
                for i in range(N_WARM):
                    nc.tensor.matmul(
                        ps_w[:], warm_sb[:, 0:P], warm_sb[:],
                        start=(i == 0), stop=(i == N_WARM - 1),
                    )
                wo = tmppool.tile([P, 4], F32, tag="wo")
                nc.vector.tensor_copy(wo[:], ps_w[:, 0:4])
                nc.sync.dma_start(warm_out[:], wo[:])

            # ---- Stage 1: hT[h, t] = silu(w1 xT) * (w3 xT), bf16 ----
            h_sb = hpool.tile([P, KH, CAP], BF16)
            for ht in range(KH):
                w1_sb, w3_sb = w1_sbs[ht], w3_sbs[ht]
                for tb in range(N_TB):
                    ps1 = pspool.tile([P, TB], F32, tag="ps1")
                    ps3 = pspool.tile([P, TB], F32, tag="ps3")
                    if ht == 0 and tb == 0:
                        # Emission order matched to head DMA arrival: ps1 k's
                        # as their x chunks land, ps3 k's after each w3 half.
                        order = (
                            [("1", k) for k in range(4)]
                            + [("3", k) for k in range(4)]
                            + [("1", k) for k in range(4, 8)]
                            + [("3", k) for k in range(4, 8)]
                            + [(g, k) for k in range(8, KD) for g in ("1", "3")]
                        )
                    else:
                        order = [(g, k) for k in range(KD) for g in ("1", "3")]
                    for g, k in order:
                        ps, w_sb = (ps1, w1_sb) if g == "1" else (ps3, w3_sb)
                        nc.tensor.matmul(
                            ps[:], w_sb[:, k], x_sb[:, k, tb],
                            start=(k == 0), stop=(k == KD - 1),
                        )
                    sil = tmppool.tile([P, TB], F32, tag="sil")
                    nc.scalar.activation(
                        sil[:], ps1[:], mybir.ActivationFunctionType.Silu
                    )
                    nc.vector.tensor_mul(h_sb[:, ht, ts(tb, TB)], sil[:], ps3[:])

            # ---- Stage 2: out[t, d] = hT.T @ w2T ----
            for tt in range(N_TT):
                for db in range(N_DB):
                    last = tt == N_TT - 1 and db == N_DB - 1
                    if last:
                        # Split the final group into 4 column sub-groups so
                        # its copies/stores pipeline under the sub-groups'
                        # matmuls instead of serializing a ~2us flush after
                        # the last MM.
                        CW = DB // 4
                        ot = opool.tile([P, DB], BF16, tag="ot")
                        for c in range(4):
                            ps = pspool.tile([P, CW], F32, tag="ps2l")
                            for k in range(KH):
                                nc.tensor.matmul(
                                    ps[:],
                                    h_sb[:, k, ts(tt, P)],
                                    w2_sbs[db][:, k, c * CW : (c + 1) * CW],
                                    start=(k == 0), stop=(k == KH - 1),
                                )
                            nc.vector.tensor_copy(ot[:, ts(c, CW)], ps[:])
                            fence = nc.sync.dma_start(
                                out[
                                    ts(tt, P),
                                    db * DB + c * CW : db * DB + (c + 1) * CW,
                                ],
                                ot[:, ts(c, CW)],
                            )
                    else:
                        ps = pspool.tile([P, DB], F32, tag="ps2")
                        for k in range(KH):
                            nc.tensor.matmul(
                                ps[:], h_sb[:, k, ts(tt, P)], w2_sbs[db][:, k],
                                start=(k == 0), stop=(k == KH - 1),
                            )
                        ot = opool.tile([P, DB], BF16, tag="ot")
                        nc.vector.tensor_copy(ot[:], ps[:])
                        nc.sync.dma_start(out[ts(tt, P), ts(db, DB)], ot[:])
            prev_fence = fence

    nc.compile()
    return nc


def _get_nc():
    global _CACHED_NC
    if _CACHED_NC is None:
        _CACHED_NC = _build_nc()
    return _CACHED_NC


def _bf16(a):
    return a.astype(mybir.dt.np(mybir.dt.bfloat16))


def _pack_inputs(x, w1, w2, w3, read_starts):
    """Per-core input dicts with DMA-optimal (partition-major) layouts."""
    in_maps = []
    for e in range(N_EXPERTS):
        s = int(read_starts[e])
        xe = x[s : s + CAP]  # [CAP, DIM]
        # xQ[tb, k2, p, j, t] = xe[tb*TB + t, (2*k2 + j)*P + p]
        xQ = np.ascontiguousarray(
            _bf16(xe).T.reshape(KD2, 2, P, N_TB, TB).transpose(3, 0, 2, 1, 4)
        )
        w1Q = np.ascontiguousarray(
            _bf16(w1[e]).T.reshape(KD, P, KH, P).transpose(2, 1, 0, 3)
        )
        w3Q = np.ascontiguousarray(
            _bf16(w3[e]).T.reshape(KD, P, KH, P).transpose(2, 1, 0, 3)
        )
        w2Q = np.ascontiguousarray(
            _bf16(w2[e]).T.reshape(KH, P, N_DB, DB).transpose(2, 1, 0, 3)
        )
        in_maps.append({"xQ": xQ, "w1Q": w1Q, "w3Q": w3Q, "w2Q": w2Q})
    return in_maps


def kernel(x, num_tokens_per_expert, w1, w2, w3):
    x = np.ascontiguousarray(np.asarray(x, dtype=np.float32))
    w1 = np.asarray(w1, dtype=np.float32)
    w2 = np.asarray(w2, dtype=np.float32)
    w3 = np.asarray(w3, dtype=np.float32)
    counts = np.asarray(num_tokens_per_expert).astype(np.int64)

    offsets = np.cumsum(counts)
    starts = offsets - counts
    # jax.lax.dynamic_slice clamps the read start so the slice is in-bounds.
    read_starts = np.clip(starts, 0, N_TOKENS - CAP)

    in_maps = _pack_inputs(x, w1, w2, w3, read_starts)
    nc = _get_nc()
    res = run_bass_kernel_spmd(nc, in_maps, core_ids=list(range(N_EXPERTS)))
    ye = [np.asarray(res.results[e]["out"], dtype=np.float32) for e in range(N_EXPERTS)]

    if np.all(counts == CAP):
        # balanced routing: per-expert tiles are disjoint and exactly cover x
        return np.concatenate(ye, axis=0)

    # general case: mask invalid slots, scatter-add to clipped positions
    y = np.zeros((N_TOKENS, DIM), np.float32)
    slot = np.arange(CAP)
    for e in range(N_EXPERTS):
        valid = slot < counts[e]
        pos = np.clip(starts[e] + slot, 0, N_TOKENS - 1)
        np.add.at(y, pos, np.where(valid[:, None], ye[e], 0.0))
    return y


# revision 24
# speedup vs baseline: 4.1546x; 1.6382x over previous
"""Grouped-experts SwiGLU MoE kernel for Trainium2 (8 NeuronCores).

Problem: x [8192, 2048] f32, 8 experts with w1/w3 [8, 1408, 2048] and
w2 [8, 2048, 1408]; tokens are expert-contiguous with a per-expert count
vector. out[t] = (silu(x_t @ w1_e.T) * (x_t @ w3_e.T)) @ w2_e.T for the
expert e owning token t.

Sharding: pure expert parallelism. Core e receives expert e's 1024-token
tile (dynamic-slice semantics of the reference) plus expert e's weights,
and computes the full SwiGLU MLP for that tile. No collectives.

Design (PE-roofline focused; per-core floor is 1056 matmuls x 512 cyc /
2.4 GHz = 225.3 us):
  - All matmul operands are bf16 (PSUM accumulates f32): same 1 cycle/row
    PE rate as f32r at free-dim 512, but half the HBM traffic. Measured
    end-to-end rel err ~3.6e-3 vs the f32 reference (gate is 2e-2).
  - One DGE queue (SP): descriptor generation (~0.63us/instr) and the
    transfer engine are effectively serial, so DMA *issue order* is the
    schedule. Head order: w1[0]/w3[0] in 256KB halves woven between the
    x chunks they gate (x rides ahead of each half so the matching ps1
    burst is never starved), then the rest of x (16 chunks of
    [128, 2, 512] = 256 KB), then the remaining weight pairs
    (slot-throttled by wpool bufs), then all of w2. Output stores are
    issued last and have the queue to themselves. Chunks below 256KB
    lose: the serial descriptor-generation rate becomes the pacer.
  - A short warmup burst of dummy matmuls (dep: one memset) runs during
    the head so the PE clock ramp completes before real work arrives.
  - Stage 1 interleaves the w1/w3 accumulations per k (emission order
    matched to chunk arrival for the first block); silu (ACT) + mul (DVE)
    write hT in bf16. Stage 2 keeps hT stationary per token-tile and
    streams w2; the final group is split into 4 column sub-groups so its
    copies/stores pipeline under the sub-groups' matmuls.
"""

from contextlib import ExitStack

import numpy as np

import concourse.bass as bass
import concourse.mybir as mybir
import concourse.tile as tile
from concourse import bacc
from concourse.bass import ts
from concourse.bass_utils import run_bass_kernel_spmd

F32 = mybir.dt.float32
BF16 = mybir.dt.bfloat16

N_TOKENS = 8192
DIM = 2048
HIDDEN = 1408
N_EXPERTS = 8
CAP = N_TOKENS // N_EXPERTS  # 1024 tokens per core
P = 128
KD = DIM // P  # 16 contraction tiles, stage 1
KD2 = KD // 2  # x DMA chunks pair two k-tiles for 1KB partition lines
KH = HIDDEN // P  # 11 contraction tiles, stage 2
TB = 512  # token-block (stage-1 moving free dim)
DB = 512  # dim-block (stage-2 moving free dim)
N_TB = CAP // TB  # 2
N_DB = DIM // DB  # 4
N_TT = CAP // P  # 8 token tiles (stage-2 stationary)
N_WARM = 6  # warmup matmuls: enough to finish the clock ramp

_CACHED_NC = None


def _build_nc(repeat=1):
    """Build the kernel; repeat>1 chains `repeat` serialized copies of the
    body (each repeat's input DMAs gated on the previous repeat's final
    store) so wall-clock slope over `repeat` measures one execution."""
    import bass_rust

    nc = bacc.Bacc("TRN2", debug=False)
    xQ = nc.dram_tensor("xQ", [N_TB, KD2, P, 2, TB], BF16, kind="ExternalInput").ap()
    w1Q = nc.dram_tensor("w1Q", [KH, P, KD, P], BF16, kind="ExternalInput").ap()
    w3Q = nc.dram_tensor("w3Q", [KH, P, KD, P], BF16, kind="ExternalInput").ap()
    w2Q = nc.dram_tensor("w2Q", [N_DB, P, KH, DB], BF16, kind="ExternalInput").ap()
    out = nc.dram_tensor("out", [CAP, DIM], BF16, kind="ExternalOutput").ap()
    warm_out = nc.dram_tensor("warm_out", [P, 4], F32, kind="ExternalOutput").ap()

    with tile.TileContext(nc) as tc, ExitStack() as ctx:
        xpool = ctx.enter_context(tc.tile_pool(name="xpool", bufs=1))
        hpool = ctx.enter_context(tc.tile_pool(name="hpool", bufs=1))
        wpool = ctx.enter_context(tc.tile_pool(name="wpool", bufs=3))
        w2pool = ctx.enter_context(tc.tile_pool(name="w2pool", bufs=4))
        warmpool = ctx.enter_context(tc.tile_pool(name="warmpool", bufs=1))
        tmppool = ctx.enter_context(tc.tile_pool(name="tmppool", bufs=3))
        opool = ctx.enter_context(tc.tile_pool(name="opool", bufs=3))
        pspool = ctx.enter_context(tc.tile_pool(name="pspool", bufs=2, space="PSUM"))

        prev_fence = None
        for rep in range(repeat):
            in_dmas = []

            def track(inst):
                in_dmas.append(inst)
                return inst

            x_sb = xpool.tile([P, KD, N_TB, TB], BF16)

            def dma_x(tb, k2):
                track(nc.sync.dma_start(x_sb[:, 2 * k2 : 2 * k2 + 2, tb], xQ[tb, k2]))

            # ---- DMA issue order defines the head schedule ----
            w1_sbs, w3_sbs = [], []
            for ht in range(KH):
                w1_sb = wpool.tile([P, KD, P], BF16, tag="w1")
                w3_sb = wpool.tile([P, KD, P], BF16, tag="w3")
                if ht == 0:
                    # First pair in halves, woven with the first x chunks so
                    # the first ps1 accumulations can issue while w3 streams;
                    # x chunk 4 rides ahead of the second halves and chunk 5
                    # between them so the ps1 k8-11 burst is never starved.
                    H2 = KD // 2
                    track(nc.sync.dma_start(w1_sb[:, 0:H2], w1Q[0, :, 0:H2]))
                    dma_x(0, 0)
                    dma_x(0, 1)
                    track(nc.sync.dma_start(w3_sb[:, 0:H2], w3Q[0, :, 0:H2]))
                    dma_x(0, 2)
                    dma_x(0, 3)
                    dma_x(0, 4)
                    track(nc.sync.dma_start(w1_sb[:, H2:KD], w1Q[0, :, H2:KD]))
                    dma_x(0, 5)
                    track(nc.sync.dma_start(w3_sb[:, H2:KD], w3Q[0, :, H2:KD]))
                    for k2 in range(6, KD2):
                        dma_x(0, k2)
                    for k2 in range(KD2):
                        dma_x(1, k2)
                else:
                    track(nc.sync.dma_start(w1_sb[:], w1Q[ht]))
                    track(nc.sync.dma_start(w3_sb[:], w3Q[ht]))
                w1_sbs.append(w1_sb)
                w3_sbs.append(w3_sb)
            # All of w2 (5.8 MB): queued behind stage-1 weights, done long
            # before stage 2 begins.
            w2_sbs = []
            for db in range(N_DB):
                w2_sb = w2pool.tile([P, KH, DB], BF16, tag="w2")
                track(nc.sync.dma_start(w2_sb[:], w2Q[db]))
                w2_sbs.append(w2_sb)

            if prev_fence is not None:
                for inst in in_dmas:
                    bass_rust.add_dep_helper(
                        inst.ins, prev_fence.ins, reason="serialize benchmark repeats"
                    )

            if rep == 0:
                # ---- PE warmup: ramp the clock while the head DMAs land ----
                warm_sb = warmpool.tile([P, TB], BF16)
                nc.vector.memset(warm_sb[:], 0.0)
                ps_w = pspool.tile([P, TB], F32, tag="ps2l")
                for i in range(N_WARM):
                    nc.tensor.matmul(
                        ps_w[:], warm_sb[:, 0:P], warm_sb[:],
                        start=(i == 0), stop=(i == N_WARM - 1),
                    )
                wo = tmppool.tile([P, 4], F32, tag="wo")
                nc.vector.tensor_copy(wo[:], ps_w[:, 0:4])
                nc.sync.dma_start(warm_out[:], wo[:])

            # ---- Stage 1: hT[h, t] = silu(w1 xT) * (w3 xT), bf16 ----
            h_sb = hpool.tile([P, KH, CAP], BF16)
            for ht in range(KH):
                w1_sb, w3_sb = w1_sbs[ht], w3_sbs[ht]
                for tb in range(N_TB):
                    ps1 = pspool.tile([P, TB], F32, tag="ps1")
                    ps3 = pspool.tile([P, TB], F32, tag="ps3")
                    if ht == 0 and tb == 0:
                        # Emission order matched to head DMA arrival: ps1 k's
                        # as their x chunks land, ps3 k's after each w3 half.
                        order = (
                            [("1", k) for k in range(4)]
                            + [("3", k) for k in range(4)]
                            + [("1", k) for k in range(4, 8)]
                            + [("3", k) for k in range(4, 8)]
                            + [("1", k) for k in range(8, 12)]
                            + [("3", k) for k in range(8, 12)]
                            + [(g, k) for k in range(12, KD) for g in ("1", "3")]
                        )
                    else:
                        order = [(g, k) for k in range(KD) for g in ("1", "3")]
                    for g, k in order:
                        ps, w_sb = (ps1, w1_sb) if g == "1" else (ps3, w3_sb)
                        nc.tensor.matmul(
                            ps[:], w_sb[:, k], x_sb[:, k, tb],
                            start=(k == 0), stop=(k == KD - 1),
                        )
                    sil = tmppool.tile([P, TB], F32, tag="sil")
                    nc.scalar.activation(
                        sil[:], ps1[:], mybir.ActivationFunctionType.Silu
                    )
                    nc.vector.tensor_mul(h_sb[:, ht, ts(tb, TB)], sil[:], ps3[:])

            # ---- Stage 2: out[t, d] = hT.T @ w2T ----
            for tt in range(N_TT):
                for db in range(N_DB):
                    last = tt == N_TT - 1 and db == N_DB - 1
                    if last:
                        # Split the final group into 4 column sub-groups so
                        # its copies/stores pipeline under the sub-groups'
                        # matmuls instead of serializing a ~2us flush after
                        # the last MM.
                        CW = DB // 4
                        ot = opool.tile([P, DB], BF16, tag="ot")
                        for c in range(4):
                            ps = pspool.tile([P, CW], F32, tag="ps2l")
                            for k in range(KH):
                                nc.tensor.matmul(
                                    ps[:],
                                    h_sb[:, k, ts(tt, P)],
                                    w2_sbs[db][:, k, c * CW : (c + 1) * CW],
                                    start=(k == 0), stop=(k == KH - 1),
                                )
                            nc.vector.tensor_copy(ot[:, ts(c, CW)], ps[:])
                            fence = nc.sync.dma_start(
                                out[
                                    ts(tt, P),
                                    db * DB + c * CW : db * DB + (c + 1) * CW,
                                ],
                                ot[:, ts(c, CW)],
                            )
                    else:
                        ps = pspool.tile([P, DB], F32, tag="ps2")
                        for k in range(KH):
                            nc.tensor.matmul(
                                ps[:], h_sb[:, k, ts(tt, P)], w2_sbs[db][:, k],
                                start=(k == 0), stop=(k == KH - 1),
                            )
                        ot = opool.tile([P, DB], BF16, tag="ot")
                        nc.vector.tensor_copy(ot[:], ps[:])
                        nc.sync.dma_start(out[ts(tt, P), ts(db, DB)], ot[:])
            prev_fence = fence

    nc.compile()
    return nc


def _get_nc():
    global _CACHED_NC
    if _CACHED_NC is None:
        _CACHED_NC = _build_nc()
    return _CACHED_NC


def _bf16(a):
    return a.astype(mybir.dt.np(mybir.dt.bfloat16))


def _pack_inputs(x, w1, w2, w3, read_starts):
    """Per-core input dicts with DMA-optimal (partition-major) layouts."""
    in_maps = []
    for e in range(N_EXPERTS):
        s = int(read_starts[e])
        xe = x[s : s + CAP]  # [CAP, DIM]
        # xQ[tb, k2, p, j, t] = xe[tb*TB + t, (2*k2 + j)*P + p]
        xQ = np.ascontiguousarray(
            _bf16(xe).T.reshape(KD2, 2, P, N_TB, TB).transpose(3, 0, 2, 1, 4)
        )
        w1Q = np.ascontiguousarray(
            _bf16(w1[e]).T.reshape(KD, P, KH, P).transpose(2, 1, 0, 3)
        )
        w3Q = np.ascontiguousarray(
            _bf16(w3[e]).T.reshape(KD, P, KH, P).transpose(2, 1, 0, 3)
        )
        w2Q = np.ascontiguousarray(
            _bf16(w2[e]).T.reshape(KH, P, N_DB, DB).transpose(2, 1, 0, 3)
        )
        in_maps.append({"xQ": xQ, "w1Q": w1Q, "w3Q": w3Q, "w2Q": w2Q})
    return in_maps


def kernel(x, num_tokens_per_expert, w1, w2, w3):
    x = np.ascontiguousarray(np.asarray(x, dtype=np.float32))
    w1 = np.asarray(w1, dtype=np.float32)
    w2 = np.asarray(w2, dtype=np.float32)
    w3 = np.asarray(w3, dtype=np.float32)
    counts = np.asarray(num_tokens_per_expert).astype(np.int64)

    offsets = np.cumsum(counts)
    starts = offsets - counts
    # jax.lax.dynamic_slice clamps the read start so the slice is in-bounds.
    read_starts = np.clip(starts, 0, N_TOKENS - CAP)

    in_maps = _pack_inputs(x, w1, w2, w3, read_starts)
    nc = _get_nc()
    res = run_bass_kernel_spmd(nc, in_maps, core_ids=list(range(N_EXPERTS)))
    ye = [np.asarray(res.results[e]["out"], dtype=np.float32) for e in range(N_EXPERTS)]

    if np.all(counts == CAP):
        # balanced routing: per-expert tiles are disjoint and exactly cover x
        return np.concatenate(ye, axis=0)

    # general case: mask invalid slots, scatter-add to clipped positions
    y = np.zeros((N_TOKENS, DIM), np.float32)
    slot = np.arange(CAP)
    for e in range(N_EXPERTS):
        valid = slot < counts[e]
        pos = np.clip(starts[e] + slot, 0, N_TOKENS - 1)
        np.add.at(y, pos, np.where(valid[:, None], ye[e], 0.0))
    return y
